# revision 9
# baseline (speedup 1.0000x reference)
"""Cross-graph node attention for Trainium2 (Bass/Tile), 8-core data parallel.

Reference (per graph b):
    Q = A @ Wq.T + bq ; K = B @ Wk.T + bk ; V = B @ Wv.T + bv
    S = Q @ K.T / sqrt(H);  S[mask==0] = -inf;  P = softmax(S, -1);  out = P @ V

Strategy (one graph per core):
  * Masked keys have exactly zero softmax weight, so the host gathers only the
    unmasked rows of B (~1030 of 2048) and pads to a multiple of 128 (NKC).
    Pad rows are killed with a -30000 additive bias inside the fused exp.
  * softmax is invariant to per-query constants, so bk drops out and
      softmax(Q K.T) == softmax(A W3 B.T + 1 (x) (u . B[k]))   with
      W3 = Wq.T Wk,  u = bq Wk.
    W3 is folded into the *key* side (B2 = B @ W3.T, NKC < NQ rows), and the
    per-key term u.B[k] rides the exp as a per-partition bias
    (exp(s + b_k) done in one ACT instruction).
  * scores run as fp8 DoubleRow matmuls (256-deep contraction, 0.5 cyc/row):
    A is shipped from host as two transposed planes A_hi (e4m3) +
    A_lo (e5m2 residual); B2 is quantized e4m3 (x8) once on the way out of
    PSUM. Two DR matmuls give ~bf16-grade scores at 2x bf16 speed.
  * exp: scores PSUM tiles are [128 keys, 2, 512] spanning two banks -- the
    pair covers two *query* halves of the same key tile, so one ACT
    instruction exp's both with a single per-partition (per-key) bias,
    writing e4m3 directly.
  * P @ V runs as fp8 DR matmuls over key-tile pairs with V split into
    V_hi (e4m3) + V_lo (e5m2 residual) planes; column H of V holds the
    softmax denominator (ones). An odd tail key tile runs in bf16.
    Output lands [query, H] in PSUM; one reciprocal + one per-partition
    multiply finish the softmax.
"""

import os
import sys

import numpy as np
import ml_dtypes

for _p in ("/opt/trn_rl_repo", "/root/.axon_site/_ro/trn_rl_repo"):
    if os.path.isdir(_p) and _p not in sys.path:
        sys.path.insert(0, _p)

import concourse.bass as bass  # noqa: E402
import concourse.tile as tile  # noqa: E402
from concourse import bacc  # noqa: E402
from concourse import mybir  # noqa: E402
from concourse.bass_utils import run_bass_kernel_spmd  # noqa: E402
from concourse.masks import make_identity  # noqa: E402

BATCH = 8
NQ = 2048
NK = 2048
H = 256
P = 128
HC = H // P  # 2 hidden chunks
QG = NQ // 1024  # 2 query-group pairs (each pair = 2 halves of 512)
SCALE = 1.0 / float(np.sqrt(H))
B2_SCALE = 8.0  # B2 quantized at x8; compensated in the ACT scale
FP32 = mybir.dt.float32
BF16 = mybir.dt.bfloat16
F8E4 = mybir.dt.float8e4
F8E5 = mybir.dt.float8e5
EXPF = mybir.ActivationFunctionType.Exp
COPYF = mybir.ActivationFunctionType.Copy
DR = mybir.MatmulPerfMode.DoubleRow
ADD = mybir.AluOpType.add
SUB = mybir.AluOpType.subtract
MULT = mybir.AluOpType.mult

NP_F8E4 = ml_dtypes.float8_e4m3
NP_F8E5 = ml_dtypes.float8_e5m2
NP_BF16 = ml_dtypes.bfloat16

MASK_NEG = -30000.0  # exp(-30000) == 0


def _build_kernel(tc, ctx, ktc, at_hi, at_lo, bt, mb, Wq, Wk, Wv, bq, bv, out):
    nc = tc.nc
    nkc = ktc * P
    npair = ktc // 2
    tail = ktc % 2

    const = ctx.enter_context(tc.tile_pool(name="const", bufs=1))
    big = ctx.enter_context(tc.tile_pool(name="big", bufs=1))
    exps = ctx.enter_context(tc.tile_pool(name="exps", bufs=2))
    expt = ctx.enter_context(tc.tile_pool(name="expt", bufs=2))
    outp = ctx.enter_context(tc.tile_pool(name="outp", bufs=4))
    small = ctx.enter_context(tc.tile_pool(name="small", bufs=4))
    dram = ctx.enter_context(tc.tile_pool(name="dram", bufs=1, space="DRAM"))
    ps_d = ctx.enter_context(tc.tile_pool(name="ps_d", bufs=2, space="PSUM"))
    ps_o = ctx.enter_context(tc.tile_pool(name="ps_o", bufs=2, space="PSUM"))

    # ---- weights / constants --------------------------------------------
    ident_bf = const.tile([P, P], BF16)
    make_identity(nc, ident_bf)
    ones_bf = const.tile([1, P], BF16)
    nc.vector.memset(ones_bf, 1.0)

    # natural-layout weight chunks W_sb[p, c, :] = W[c*128 + p, :], cast-DMA'd
    Wq_bf = const.tile([P, HC, H], BF16, tag="wq_bf")
    Wk_bf = const.tile([P, HC, H], BF16, tag="wk_bf")
    Wv_bf = const.tile([P, HC, H], BF16, tag="wv_bf")
    for w_dram, w_sb in ((Wq, Wq_bf), (Wk, Wk_bf), (Wv, Wv_bf)):
        nc.gpsimd.dma_start(w_sb, w_dram.rearrange("(c p) h -> p c h", p=P))

    # bq scaled by 1/sqrt(H): bqs[p, c] = SCALE * bq[c*128 + p]
    bq_f32 = small.tile([P, HC], FP32, tag="bq_f32")
    nc.sync.dma_start(bq_f32, bq.rearrange("(c p) -> p c", p=P))
    bqs_bf = const.tile([P, HC], BF16, tag="bqs_bf")
    nc.vector.tensor_scalar(bqs_bf, bq_f32, SCALE, None, MULT)

    bv_bf = const.tile([1, H], BF16, tag="bv_bf")
    nc.gpsimd.dma_start(bv_bf, bv[None, :])

    # mask bias mb_sb[p, c] = mb[c*128 + p] (0 for real keys, -30000 for pad)
    mb_sb = const.tile([P, ktc], FP32, tag="mb")
    nc.sync.dma_start(mb_sb, mb.rearrange("(c p) -> p c", p=P))

    # W3T[hin, h'] = (Wk.T Wq)[hin, h']; chunks W3T_bf[p, c, :] = W3T[c*128+p, :]
    W3T_bf = const.tile([P, HC, H], BF16, tag="w3t_bf")
    for c in range(HC):
        pw = ps_o.tile([P, 512], FP32, tag="po")
        for tc_ in range(HC):
            nc.tensor.matmul(
                pw[:, :H],
                lhsT=Wk_bf[:, tc_, c * P : (c + 1) * P],
                rhs=Wq_bf[:, tc_, :],
                start=(tc_ == 0),
                stop=(tc_ == HC - 1),
            )
        nc.vector.tensor_copy(W3T_bf[:, c, :], pw[:, :H])

    # u_s = (SCALE*bq) @ Wk as columns u_col[p, c] = u_s[c*128 + p]
    pu = ps_o.tile([P, 512], FP32, tag="po")
    for tc_ in range(HC):
        nc.tensor.matmul(
            pu[:1, :H],
            lhsT=bqs_bf[:, tc_ : tc_ + 1],
            rhs=Wk_bf[:, tc_, :],
            start=(tc_ == 0),
            stop=(tc_ == HC - 1),
        )
    u_row = small.tile([1, H], BF16, tag="u_row")
    nc.vector.tensor_copy(u_row, pu[:1, :H])
    # redistribute u across partitions via a DRAM roundtrip
    u_dram = dram.tile([H], BF16, tag="u_dram")
    nc.sync.dma_start(u_dram[None, :], u_row)
    u_col = const.tile([P, HC], BF16, tag="u_col")
    nc.sync.dma_start(u_col, u_dram.rearrange("(c p) -> p c", p=P))

    # WvT[p, c, :] = Wv.T[c*128 + p, :]
    WvT_bf = const.tile([P, HC, H], BF16, tag="wvt_bf")
    for c in range(HC):
        pw = ps_o.tile([P, 256], BF16, tag="po_bf")
        for m in range(HC):
            nc.tensor.transpose(
                pw[:, m * P : (m + 1) * P],
                Wv_bf[:, m, c * P : (c + 1) * P],
                ident_bf,
            )
        nc.vector.tensor_copy(WvT_bf[:, c, :], pw[:, :H])

    # ---- A planes / B load ----------------------------------------------
    AT_hi = big.tile([P, HC, NQ], F8E4, tag="at_hi")
    AT_lo = big.tile([P, HC, NQ], F8E5, tag="at_lo")
    BT_bf = big.tile([P, HC, nkc], BF16, tag="bt")
    nc.sync.dma_start(BT_bf, bt.rearrange("(c p) n -> p c n", p=P))
    for g in range(QG):
        qs = slice(g * 1024, (g + 1) * 1024)
        nc.sync.dma_start(AT_hi[:, :, qs], at_hi.rearrange("(c p) n -> p c n", p=P)[:, :, qs])
        nc.sync.dma_start(AT_lo[:, :, qs], at_lo.rearrange("(c p) n -> p c n", p=P)[:, :, qs])

    # ---- per-key bias: bias_col[p, kt] = u_s . B[k] + mb ----------------
    pb = ps_o.tile([P, 512], FP32, tag="po")
    for kt in range(ktc):
        for c in range(HC):
            nc.tensor.matmul(
                pb[:, kt : kt + 1],
                lhsT=BT_bf[:, c, kt * P : (kt + 1) * P],
                rhs=u_col[:, c : c + 1],
                start=(kt == 0 and c == 0),
                stop=(kt == ktc - 1 and c == HC - 1),
                skip_group_check=True,
            )
    bias_col = const.tile([P, ktc], FP32, tag="bias_col")
    nc.vector.tensor_tensor(bias_col, pb[:, :ktc], mb_sb, ADD)

    # ---- B2T = W3T.T-chunks @ BT, quantized e4m3 x8 ----------------------
    B2T_f8 = const.tile([P, HC, nkc], F8E4, tag="b2t")
    n_kch = (nkc + 511) // 512
    for m in range(HC):
        for c in range(n_kch):
            ks = slice(c * 512, min((c + 1) * 512, nkc))
            w = ks.stop - ks.start
            pw = ps_o.tile([P, 512], FP32, tag="po")
            for hc_in in range(HC):
                nc.tensor.matmul(
                    pw[:, :w],
                    lhsT=W3T_bf[:, hc_in, m * P : (m + 1) * P],
                    rhs=BT_bf[:, hc_in, ks],
                    start=(hc_in == 0),
                    stop=(hc_in == HC - 1),
                )
            nc.vector.tensor_scalar(B2T_f8[:, m, ks], pw[:, :w], B2_SCALE, None, MULT)

    # ---- V planes: V_hi (e4m3) + V_lo (e5m2), ones column = denominator --
    NV = H + 1  # 257
    V_hi = const.tile([P, npair, HC, NV], F8E4, tag="v_hi", name="v_hi") if npair else None
    V_lo = const.tile([P, npair, HC, NV], F8E5, tag="v_lo", name="v_lo") if npair else None
    V_bt = const.tile([P, NV], BF16, tag="v_bt", name="v_bt") if tail else None

    def v_matmuls(dst, kt):
        for c in range(HC):
            nc.tensor.matmul(
                dst,
                lhsT=BT_bf[:, c, kt * P : (kt + 1) * P],
                rhs=WvT_bf[:, c, :],
                start=(c == 0),
                stop=False,
            )
        nc.tensor.matmul(dst, lhsT=ones_bf[:1, :], rhs=bv_bf, start=False, stop=True)

    for t in range(npair):
        pv = ps_d.tile([P, HC, 512], FP32, tag="pd")
        for i in range(HC):
            v_matmuls(pv[:, i, :H], 2 * t + i)
        nc.scalar.activation(V_hi[:, t, :, :H], pv[:, :, :H], COPYF, scale=1.0)
        nc.vector.tensor_tensor(V_lo[:, t, :, :H], pv[:, :, :H], V_hi[:, t, :, :H], SUB)
    if tail:
        pv = ps_o.tile([P, 512], FP32, tag="po")
        v_matmuls(pv[:, :H], ktc - 1)
        nc.vector.tensor_copy(V_bt[:, :H], pv[:, :H])
        nc.gpsimd.memset(V_bt[:, H : H + 1], 1.0)
    if npair:
        nc.gpsimd.memset(V_hi[:, :, :, H : H + 1], 1.0)
        nc.gpsimd.memset(V_lo[:, :, :, H : H + 1], 0.0)

    # ---- main loop: scores -> exp -> PV, pipelined over query-group pairs -
    act_scale = SCALE / B2_SCALE

    def emit_scores(g, kt, e8, e_bt):
        ps = ps_d.tile([P, HC, 512], FP32, tag="pd")
        for gh in range(2):
            qs = slice(g * 1024 + gh * 512, g * 1024 + gh * 512 + 512)
            for pi, plane in enumerate((AT_hi, AT_lo)):
                nc.tensor.matmul(
                    ps[:, gh, :],
                    lhsT=B2T_f8[:, :, kt * P : (kt + 1) * P],
                    rhs=plane[:, :, qs],
                    start=(pi == 0),
                    stop=(pi == 1),
                    perf_mode=DR,
                )
        dst = e8[:, kt, :, :] if kt < 2 * npair else e_bt
        nc.scalar.activation(dst, ps, EXPF, bias=bias_col[:, kt : kt + 1], scale=act_scale)

    def emit_pv(g, gh, j, e8, e_bt):
        po = ps_o.tile([P, 512], FP32, tag="po")
        js = slice(j * P, (j + 1) * P)
        for t in range(npair):
            lhsT = e8[:, 2 * t : 2 * t + 2, gh, js]
            nc.tensor.matmul(po[:, :NV], lhsT=lhsT, rhs=V_hi[:, t], start=(t == 0), stop=False, perf_mode=DR)
            nc.tensor.matmul(po[:, :NV], lhsT=lhsT, rhs=V_lo[:, t], start=False, stop=(False if tail else t == npair - 1), perf_mode=DR)
        if tail:
            nc.tensor.matmul(po[:, :NV], lhsT=e_bt[:, gh, js], rhs=V_bt, start=(npair == 0), stop=True)
        rec = small.tile([P, 1], FP32, tag="rec")
        nc.vector.reciprocal(rec, po[:, H : H + 1])
        ot = outp.tile([P, H], FP32, tag="ot")
        nc.vector.tensor_scalar(ot, po[:, :H], rec, None, MULT)
        row0 = g * 1024 + gh * 512 + j * P
        nc.sync.dma_start(out[row0 : row0 + P, :], ot)

    e_tiles = []
    pv_queue = []
    for g in range(QG):
        e8 = exps.tile([P, max(2 * npair, 1), HC, 512], F8E4, tag="e8")
        e_bt = expt.tile([P, HC, 512], BF16, tag="e_bt", name="e_bt") if tail else None
        e_tiles.append((e8, e_bt))
        for kt in range(ktc):
            emit_scores(g, kt, e8, e_bt)
            # interleave PV of the previous group pair to keep PE busy
            if pv_queue:
                emit_pv(*pv_queue.pop(0))
        for gh in range(2):
            for j in range(4):
                pv_queue.append((g, gh, j, e8, e_bt))
    while pv_queue:
        emit_pv(*pv_queue.pop(0))


_NC_CACHE = {}
_LAST_KTC = [9]


def build_nc(ktc=None):
    if ktc is None:
        ktc = _LAST_KTC[0]
    if ktc in _NC_CACHE:
        return _NC_CACHE[ktc]
    nkc = ktc * P
    nc = bacc.Bacc("TRN2", target_bir_lowering=False, debug=False)
    aps = {}
    for name, shape, dt in (
        ("at_hi", [H, NQ], F8E4),
        ("at_lo", [H, NQ], F8E5),
        ("bt", [H, nkc], BF16),
        ("mb", [nkc], FP32),
        ("Wq", [H, H], FP32),
        ("Wk", [H, H], FP32),
        ("Wv", [H, H], FP32),
        ("bq", [H], FP32),
        ("bv", [H], FP32),
    ):
        aps[name] = nc.dram_tensor(name, shape, dt, kind="ExternalInput").ap()
    out_ap = nc.dram_tensor("out", [NQ, H], FP32, kind="ExternalOutput").ap()

    from contextlib import ExitStack

    with tile.TileContext(nc) as tc, ExitStack() as ctx:
        _build_kernel(
            tc, ctx, ktc,
            aps["at_hi"], aps["at_lo"], aps["bt"], aps["mb"],
            aps["Wq"], aps["Wk"], aps["Wv"], aps["bq"], aps["bv"],
            out_ap,
        )
    nc.compile()
    _NC_CACHE[ktc] = nc
    _LAST_KTC[0] = ktc
    return nc


def make_in_maps(A, B, mask_B, Wq, Wk, Wv, bq, bv):
    A = np.ascontiguousarray(np.asarray(A, dtype=np.float32))
    B = np.ascontiguousarray(np.asarray(B, dtype=np.float32))
    mask_B = np.asarray(mask_B)
    Wq = np.ascontiguousarray(np.asarray(Wq, dtype=np.float32))
    Wk = np.ascontiguousarray(np.asarray(Wk, dtype=np.float32))
    Wv = np.ascontiguousarray(np.asarray(Wv, dtype=np.float32))
    bq = np.ascontiguousarray(np.asarray(bq, dtype=np.float32))
    bv = np.ascontiguousarray(np.asarray(bv, dtype=np.float32))

    counts = [int(np.count_nonzero(mask_B[b])) for b in range(BATCH)]
    ktc = max(1, int(np.ceil(max(counts) / P)))
    nkc = ktc * P

    in_maps = []
    for b in range(BATCH):
        idx = np.nonzero(mask_B[b])[0]
        c = len(idx)
        Bc = np.zeros((nkc, H), np.float32)
        Bc[:c] = B[b][idx]
        mb = np.full((nkc,), MASK_NEG, np.float32)
        mb[:c] = 0.0

        a = A[b]
        a_hi = a.astype(NP_F8E4)
        a_lo = (a - a_hi.astype(np.float32)).astype(NP_F8E5)
        at_hi = np.ascontiguousarray(a_hi.T)
        at_lo = np.ascontiguousarray(a_lo.T)
        bt = np.ascontiguousarray(Bc.astype(NP_BF16).T)

        in_maps.append(
            {
                "at_hi": at_hi,
                "at_lo": at_lo,
                "bt": bt,
                "mb": mb,
                "Wq": Wq,
                "Wk": Wk,
                "Wv": Wv,
                "bq": bq,
                "bv": bv,
            }
        )
    return in_maps, ktc


def run(inputs, trace=False):
    in_maps, ktc = make_in_maps(
        inputs["A"], inputs["B"], inputs["mask_B"],
        inputs["Wq"], inputs["Wk"], inputs["Wv"], inputs["bq"], inputs["bv"],
    )
    nc = build_nc(ktc)
    res = run_bass_kernel_spmd(nc, in_maps, core_ids=list(range(BATCH)), trace=trace)
    out = np.stack([res.results[b]["out"] for b in range(BATCH)], axis=0)
    return out.astype(np.float32), res


def kernel(A, B, mask_B, Wq, bq, Wk, bk, Wv, bv):
    # bk is unused: softmax is invariant to the per-query bk terms
    out, _ = run(
        {
            "A": A, "B": B, "mask_B": mask_B,
            "Wq": Wq, "bq": bq, "Wk": Wk, "Wv": Wv, "bv": bv,
        }
    )
    return out
